# revision 1
# baseline (speedup 1.0000x reference)
"""Data-parallel 3x3 conv (implicit GEMM) for Trainium2, 8 NeuronCores.

Problem: x (32,128,56,56) f32, W (256,1152) f32 [C_out, C_in*KH*KW, taps in
(ci,kh,kw) order], b (256,), stride 1, pad 1 -> out (32,256,56,56) f32.

Strategy
- Shard the batch dim across the 8 cores (4 images each); replicate W and b.
- Host-side prep: zero-pad x to 58x58 (so every shifted read in the kernel is
  a plain strided AP, no edge cases), and pre-transpose W into the stationary
  [ci, (tap, co)] layout the tensor engine wants.
- Per core: keep the whole padded shard (6.9 MB) + weights in SBUF. For each
  (image, 8-row block, co-half): accumulate 9 shifted matmuls (one per tap)
  into one PSUM bank, contraction dim = C_in = 128 (full partition width),
  moving dim N = 8*56 = 448. Weights/activations are loaded as float32r via
  casting SWDGE DMAs (full 1 cycle/row PE rate at N>=256, vs 4 cycles/row for
  plain fp32). ScalarE fuses the bias-add with the PSUM->SBUF copy, and the
  result DMAs out from ScalarE's HWDGE ring.
"""

import numpy as np

import bass_rust as _br
import concourse.bass as bass
import concourse.mybir as mybir
import concourse.tile as tile
from concourse.bass_utils import run_bass_kernel_spmd

N_CORES = 8
B, C_IN, H, W_ = 32, 128, 56, 56
C_OUT = 256
B_LOC = B // N_CORES          # 4 images per core
HP, WP = H + 2, W_ + 2        # padded 58x58
IMG_PAD = HP * WP             # 3364
ROWS_PER_BLK = 8              # 8 rows * 56 cols = 448 = moving dim (<=512 f32)
N_BLK = H // ROWS_PER_BLK     # 7
N_MOV = ROWS_PER_BLK * W_     # 448
N_TAPS = 9
CO_TILES = C_OUT // 128       # 2

_F32 = mybir.dt.float32
_F32R = mybir.dt.float32r


def _split_multi_waits(nc):
    """This walrus build accepts at most ONE sync-wait per instruction.

    Tile can emit several (e.g. a matmul waiting on two input DMAs, or the
    exit drain waiting on every outstanding semaphore). Hoist the extras onto
    injected same-engine NOPs immediately ahead of the offender — sequencers
    execute their stream in order, so the waits still all happen before it.
    """
    for bb in nc.m.functions[0].blocks:
        il = bb.instructions
        i = 0
        while i < len(il):
            inst = il[i]
            si = inst.sync_info
            w = list(si.on_wait) if (si and si.on_wait) else []
            if len(w) > 1:
                si.on_wait = w[-1:]
                for wi in w[:-1]:
                    nop = mybir.InstNoOp(
                        name=nc.get_next_instruction_name(), ins=[], outs=[]
                    )
                    nop.engine = inst.engine
                    nop.sync_info = _br.SyncInfo(on_wait=[wi], on_update=[])
                    nc.register_instruction(nop)
                    il.insert(i, nop)
                    i += 1
            i += 1


def _build_program():
    nc = bass.Bass("TRN2", target_bir_lowering=False, debug=False,
                   num_devices=N_CORES)
    xp = nc.dram_tensor("xp", [B_LOC, C_IN, IMG_PAD], _F32,
                        kind="ExternalInput").ap()
    wt = nc.dram_tensor("wt", [C_IN, N_TAPS * C_OUT], _F32,
                        kind="ExternalInput").ap()
    bt = nc.dram_tensor("bt", [128, CO_TILES], _F32, kind="ExternalInput").ap()
    out = nc.dram_tensor("out", [B_LOC, C_OUT, H, W_], _F32,
                         kind="ExternalOutput").ap()

    with tile.TileContext(nc) as tc:
        with (
            tc.tile_pool(name="xpool", bufs=1) as xpool,
            tc.tile_pool(name="wpool", bufs=1) as wpool,
            tc.tile_pool(name="opool", bufs=4) as opool,
            tc.tile_pool(name="wmpool", bufs=1, space="PSUM") as wmpool,
            tc.tile_pool(name="ppool", bufs=4, space="PSUM") as ppool,
        ):
            # PE warm-up: the HAM clock-gate runs the PE at 1.2 GHz until it
            # sees ~3.4 us of sustained activity. The real matmul stream can't
            # start until the weight/activation loads land (~8 us), so burn
            # that window on dummy matmuls over a zeroed scratch tile into a
            # never-read PSUM bank — the real stream then starts at 2.4 GHz
            # (cost model: 111.4 -> 107.4 us).
            scratch = wpool.tile([128, 256], _F32, tag="scratch")
            nc.gpsimd.memset(scratch[:], 0.0)
            wps = wmpool.tile([128, 256], _F32, tag="wps")
            for _ in range(8):
                nc.tensor.matmul(wps[:], scratch[:, :128], scratch[:],
                                 start=True, stop=True)
            # f32 -> f32r casting loads must go through SWDGE (gpsimd).
            # Split W in two and give image 0 a small leading chunk so the
            # first matmul group isn't serialized behind whole-tensor loads
            # (cost model: 113.8 -> 111.4 us).
            w_sb = wpool.tile([C_IN, N_TAPS * C_OUT], _F32R, tag="w")
            wcols = N_TAPS * C_OUT
            nc.gpsimd.dma_start(w_sb[:, :wcols // 2], wt[:, :wcols // 2])
            nc.gpsimd.dma_start(w_sb[:, wcols // 2:], wt[:, wcols // 2:])
            b_sb = wpool.tile([128, CO_TILES], _F32, tag="b")
            nc.sync.dma_start(b_sb[:], bt[:])

            x_sb = []
            for n in range(B_LOC):
                t_ = xpool.tile([C_IN, IMG_PAD], _F32R, tag=f"x{n}")
                if n == 0:
                    bounds = [0, IMG_PAD // 16, IMG_PAD // 4, IMG_PAD // 2,
                              3 * IMG_PAD // 4, IMG_PAD]
                else:
                    bounds = [0, IMG_PAD // 4, IMG_PAD // 2,
                              3 * IMG_PAD // 4, IMG_PAD]
                for lo, hi in zip(bounds[:-1], bounds[1:]):
                    nc.gpsimd.dma_start(t_[:, lo:hi], xp[n][:, lo:hi])
                x_sb.append(t_)

            for n in range(B_LOC):
                xv = x_sb[n][:].rearrange("p (h w) -> p h w", h=HP, w=WP)
                for j in range(N_BLK):
                    for t in range(CO_TILES):
                        ps = ppool.tile([128, N_MOV], _F32, tag="ps")
                        for k in range(N_TAPS):
                            kh, kw = divmod(k, 3)
                            r0 = j * ROWS_PER_BLK + kh
                            rhs = xv[:, r0:r0 + ROWS_PER_BLK, kw:kw + W_]
                            lhsT = w_sb[:, k * C_OUT + t * 128:
                                        k * C_OUT + t * 128 + 128]
                            nc.tensor.matmul(
                                ps[:], lhsT, rhs,
                                start=(k == 0),
                                stop=(k == N_TAPS - 1),
                            )
                        o_sb = opool.tile([128, N_MOV], _F32, tag="o")
                        nc.scalar.activation(
                            o_sb[:], ps[:],
                            mybir.ActivationFunctionType.Identity,
                            bias=b_sb[:, t:t + 1],
                        )
                        nc.scalar.dma_start(
                            out[n, bass.ts(t, 128), bass.ts(j, ROWS_PER_BLK), :],
                            o_sb[:],
                        )

    _split_multi_waits(nc)
    return nc


_CACHED_NC = None


def _get_program():
    global _CACHED_NC
    if _CACHED_NC is None:
        _CACHED_NC = _build_program()
    return _CACHED_NC


def _prep_inputs(x, W, b):
    xp_all = np.pad(x, ((0, 0), (0, 0), (1, 1), (1, 1)))
    wt = np.ascontiguousarray(
        W.reshape(C_OUT, C_IN, N_TAPS).transpose(1, 2, 0).reshape(C_IN, -1)
    )
    bt = np.ascontiguousarray(b.reshape(CO_TILES, 128).T)
    in_maps = []
    for i in range(N_CORES):
        shard = np.ascontiguousarray(
            xp_all[i * B_LOC:(i + 1) * B_LOC].reshape(B_LOC, C_IN, IMG_PAD)
        )
        in_maps.append({"xp": shard, "wt": wt, "bt": bt})
    return in_maps


def kernel(x, W, b):
    x = np.asarray(x, dtype=np.float32)
    W = np.asarray(W, dtype=np.float32)
    b = np.asarray(b, dtype=np.float32)
    nc = _get_program()
    in_maps = _prep_inputs(x, W, b)
    res = run_bass_kernel_spmd(nc, in_maps, list(range(N_CORES)), trace=False)
    return np.concatenate([res.results[i]["out"] for i in range(N_CORES)], axis=0)



# revision 6
# speedup vs baseline: 1.3432x; 1.3432x over previous
"""Data-parallel 3x3 conv (implicit GEMM) for Trainium2, 8 NeuronCores.

Problem: x (32,128,56,56) f32, W (256,1152) f32 [C_out, C_in*KH*KW, taps in
(ci,kh,kw) order], b (256,), stride 1, pad 1 -> out (32,256,56,56) f32.

Strategy (fp8 DoubleRow implicit GEMM)
- Shard the batch dim across the 8 cores (4 images each); replicate W and b.
- The tensor engine's fp8 DoubleRow mode contracts 256 elements per
  instruction at 0.5 cycles/row - 4x the per-row FLOP rate of f32r. Inputs
  are decomposed host-side into e4m3 hi+lo pairs (x ~ x8+r8, 32*W ~ w8+s8;
  the 32x weight scale keeps w8/s8 out of e4m3's coarse subnormal range) and
  the conv is computed as x8@w8 (all 9 taps) + r8@w8 (8 taps) + x8@s8
  (7 taps) = 24 slot products = 12 DoubleRow matmuls per output tile,
  accumulated in one PSUM bank. Dropping 1 r-tap + 2 s-taps trades
  rel-err 1.3e-3 -> 1.44e-2 (gate 2e-2) for 14->12 instructions.
- Moving dim: flat padded-row span of 464 = 8 rows x 58 cols per block
  (the 2 pad columns per row are computed and discarded host-side), so every
  tap read is a contiguous [128, 2, 464] AP with a constant pair stride.
- ScalarE fuses bias-add and the 1/32 weight-scale compensation with the
  PSUM->SBUF copy, writing fp16 into a per-(image,cotile) staging tile that
  DMAs out as one contiguous descriptor per partition (f32 output would
  double DMA-out bytes; the host casts back to f32).
"""

import numpy as np
import ml_dtypes

import bass_rust as _br
import concourse.bass as bass
import concourse.mybir as mybir
import concourse.tile as tile
from concourse.bass_utils import run_bass_kernel_spmd

N_CORES = 8
B, C_IN, H, W_ = 32, 128, 56, 56
C_OUT = 256
B_LOC = B // N_CORES          # 4 images per core
HP, WP = H + 2, W_ + 2        # padded 58x58
IMG_PAD = HP * WP             # 3364
PLANE = 3392                  # x8/r8 plane pitch in SBUF/DRAM (pad to 64B mult)
N_PLANES = 3                  # x8 | r8 | x8-copy
XBYTES = N_PLANES * PLANE     # 10176 bytes/partition/image
ROWS_PER_BLK = 8
N_BLK = H // ROWS_PER_BLK     # 7
NB = ROWS_PER_BLK * WP        # 464 moving elements per block (58-wide rows)
OUTSPAN = N_BLK * NB          # 3248 = 56*58
CO_TILES = C_OUT // 128       # 2
N_TAPS = 9
W_SCALE = 32.0                # keeps w8/s8 out of e4m3 subnormal range

_F32 = mybir.dt.float32
_F16 = mybir.dt.float16
_U8 = mybir.dt.uint8
_F8 = mybir.dt.float8e4
_E4M3 = ml_dtypes.float8_e4m3

# tap k = kh*3 + kw reads padded offset sh(k) = kh*58 + kw
_SH = [kh * WP + kw for kh in range(3) for kw in range(3)]

# 24 slots, each (is_s_weight, tap, plane): plane 0 = x8, 1 = r8, 2 = x8
# again (duplicate copy). M_k = (w8[k] vs x8), R_k = (w8[k] vs r8),
# S_k = (s8[k] vs x8). Drop R_4, S_1, S_7 (rel err 1.44e-2 < 2e-2 gate).
# HW constraint (probed): the DoubleRow ifmap pair stride must be
# non-overlapping (|stride| >= moving size), so every pair spans two
# different planes - that's what the duplicate x8 plane is for.
_PAIRS = [
    # (slotA, slotB) with slot = (s_weight?, tap, plane)
    ((False, 0, 0), (False, 0, 1)),  # M0, R0
    ((False, 1, 0), (False, 1, 1)),  # M1, R1
    ((False, 2, 0), (False, 2, 1)),  # M2, R2
    ((False, 3, 0), (False, 3, 1)),  # M3, R3
    ((False, 4, 0), (False, 5, 1)),  # M4, R5
    ((False, 5, 0), (False, 6, 1)),  # M5, R6
    ((False, 6, 0), (False, 7, 1)),  # M6, R7
    ((False, 7, 0), (False, 8, 1)),  # M7, R8
    ((False, 8, 0), (True, 0, 2)),   # M8, S0
    ((True, 2, 0), (True, 3, 2)),    # S2, S3
    ((True, 4, 0), (True, 5, 2)),    # S4, S5
    ((True, 6, 0), (True, 8, 2)),    # S6, S8
]
N_PAIRS = len(_PAIRS)          # 12
WT_COLS = CO_TILES * N_PAIRS * 2 * 128   # 6144 bytes/partition


def _slot_off(slot):
    _, tap, plane = slot
    return plane * PLANE + _SH[tap]


def _split_multi_waits(nc):
    """This walrus build accepts at most ONE sync-wait per instruction.

    Tile can emit several (e.g. a matmul waiting on two input DMAs, or the
    exit drain waiting on every outstanding semaphore). Hoist the extras onto
    injected same-engine NOPs immediately ahead of the offender - sequencers
    execute their stream in order, so the waits still all happen before it.
    """
    for bb in nc.m.functions[0].blocks:
        il = bb.instructions
        i = 0
        while i < len(il):
            inst = il[i]
            si = inst.sync_info
            w = list(si.on_wait) if (si and si.on_wait) else []
            if len(w) > 1:
                si.on_wait = w[-1:]
                for wi in w[:-1]:
                    nop = mybir.InstNoOp(
                        name=nc.get_next_instruction_name(), ins=[], outs=[]
                    )
                    nop.engine = inst.engine
                    nop.sync_info = _br.SyncInfo(on_wait=[wi], on_update=[])
                    nc.register_instruction(nop)
                    il.insert(i, nop)
                    i += 1
            i += 1


def _build_program():
    nc = bass.Bass("TRN2", target_bir_lowering=False, debug=False,
                   num_devices=N_CORES)
    xp = nc.dram_tensor("xp", [B_LOC, C_IN, N_PLANES, PLANE], _U8,
                        kind="ExternalInput").ap()
    wt = nc.dram_tensor("wt", [C_IN, WT_COLS], _U8, kind="ExternalInput").ap()
    bt = nc.dram_tensor("bt", [128, CO_TILES], _F32, kind="ExternalInput").ap()
    out = nc.dram_tensor("out", [B_LOC, CO_TILES, 128, OUTSPAN], _F16,
                         kind="ExternalOutput").ap()

    with tile.TileContext(nc) as tc:
        with (
            tc.tile_pool(name="xpool", bufs=1) as xpool,
            tc.tile_pool(name="wpool", bufs=1) as wpool,
            tc.tile_pool(name="opool", bufs=3) as opool,
            tc.tile_pool(name="wmpool", bufs=1, space="PSUM") as wmpool,
            tc.tile_pool(name="ppool", bufs=4, space="PSUM") as ppool,
        ):
            # PE warm-up: the p-state model runs the PE below 2.4 GHz until it
            # sees ~3 us of continuous activity. The real matmul stream can't
            # start until the weight/x loads land (~4.5 us), so burn that
            # window on dummy matmuls into a never-read PSUM bank.
            scratch = wpool.tile([128, 256], _F32, tag="scratch")
            nc.gpsimd.memset(scratch[:], 0.0)
            wps = wmpool.tile([128, 256], _F32, tag="wps")
            for _ in range(10):
                nc.tensor.matmul(wps[:], scratch[:, :128], scratch[:],
                                 start=True, stop=True)

            # weights: cotile 0 first so group (n=0,t=0,j=0) starts ASAP
            w_sb = wpool.tile([C_IN, WT_COLS], _U8, tag="w")
            half = WT_COLS // 2
            nc.sync.dma_start(w_sb[:, :half], wt[:, :half])
            b_sb = wpool.tile([128, CO_TILES], _F32, tag="b")
            nc.sync.dma_start(b_sb[:], bt[:])
            nc.sync.dma_start(w_sb[:, half:], wt[:, half:])

            # x planes: image 0 head rows first (2-plane strided chunk)
            HEAD = 640   # first 11 padded rows per plane
            x_sb = []
            for n in range(B_LOC):
                t_ = xpool.tile([C_IN, N_PLANES, PLANE], _U8, tag=f"x{n}")
                if n == 0:
                    nc.gpsimd.dma_start(t_[:, :, :HEAD], xp[n][:, :, :HEAD])
                    nc.gpsimd.dma_start(t_[:, :, HEAD:], xp[n][:, :, HEAD:])
                else:
                    nc.gpsimd.dma_start(t_[:], xp[n][:])
                x_sb.append(t_)

            def rhs_ap(n, j, p):
                pair = _PAIRS[p]
                base = x_sb[n][:]
                off_a = _slot_off(pair[0]) + j * NB
                delta = _slot_off(pair[1]) - _slot_off(pair[0])
                ap = bass.AP(base.tensor, base.offset + off_a,
                             [list(base.ap[0]), [delta, 2], [1, NB]])
                return ap.bitcast(_F8)

            def lhsT_ap(t, p):
                off = (t * N_PAIRS + p) * 256
                return (w_sb[:, off:off + 256].bitcast(_F8)
                        .rearrange("p (a b) -> p a b", a=2))

            for n in range(B_LOC):
                for t in range(CO_TILES):
                    o_img = opool.tile([128, OUTSPAN], _F16, tag="o")
                    for j in range(N_BLK):
                        ps = ppool.tile([128, NB], _F32, tag="ps")
                        for p in range(N_PAIRS):
                            nc.tensor.matmul(
                                ps[:], lhsT_ap(t, p), rhs_ap(n, j, p),
                                start=(p == 0),
                                stop=(p == N_PAIRS - 1),
                                perf_mode=mybir.MatmulPerfMode.DoubleRow,
                            )
                        nc.scalar.activation(
                            o_img[:, j * NB:(j + 1) * NB], ps[:],
                            mybir.ActivationFunctionType.Identity,
                            bias=b_sb[:, t:t + 1],
                            scale=1.0 / W_SCALE,
                        )
                        # split the store so the drain tail after the last
                        # group is short
                        if j == 4:
                            nc.scalar.dma_start(out[n, t][:, :5 * NB],
                                                o_img[:, :5 * NB])
                        elif j == N_BLK - 1:
                            nc.scalar.dma_start(out[n, t][:, 5 * NB:],
                                                o_img[:, 5 * NB:])

    _split_multi_waits(nc)
    return nc


_CACHED_NC = None


def _get_program():
    global _CACHED_NC
    if _CACHED_NC is None:
        _CACHED_NC = _build_program()
    return _CACHED_NC


def _prep_inputs(x, W, b):
    xp_all = np.pad(x, ((0, 0), (0, 0), (1, 1), (1, 1))).reshape(B, C_IN, IMG_PAD)
    x8 = xp_all.astype(_E4M3)
    r8 = (xp_all - x8.astype(np.float32)).astype(_E4M3)
    xi = np.zeros((B, C_IN, N_PLANES, PLANE), np.uint8)
    xi[:, :, 0, :IMG_PAD] = x8.view(np.uint8)
    xi[:, :, 1, :IMG_PAD] = r8.view(np.uint8)
    xi[:, :, 2, :IMG_PAD] = x8.view(np.uint8)

    # taps: W is [C_out, C_in*3*3] flattened in (ci, kh, kw) order
    w4 = (W * W_SCALE).reshape(C_OUT, C_IN, N_TAPS)
    w8 = w4.astype(_E4M3)
    s8 = (w4 - w8.astype(np.float32)).astype(_E4M3)
    # wt layout: [ci, cot, pair, slot, co_local] bytes
    wtb = np.zeros((C_IN, CO_TILES, N_PAIRS, 2, 128), np.uint8)
    for t in range(CO_TILES):
        co = slice(t * 128, (t + 1) * 128)
        for p, pair in enumerate(_PAIRS):
            for s_i, (is_s, tap, _plane) in enumerate(pair):
                blk = (s8 if is_s else w8)[co, :, tap]      # [co_loc, ci] e4m3
                wtb[:, t, p, s_i, :] = np.ascontiguousarray(blk.T).view(np.uint8)
    wt = wtb.reshape(C_IN, WT_COLS)

    bt = np.ascontiguousarray(b.reshape(CO_TILES, 128).T)
    in_maps = []
    for i in range(N_CORES):
        shard = np.ascontiguousarray(xi[i * B_LOC:(i + 1) * B_LOC])
        in_maps.append({"xp": shard, "wt": wt, "bt": bt})
    return in_maps


def _postprocess(res_concat):
    # res_concat: [B, CO_TILES, 128, OUTSPAN] f16
    o = res_concat.reshape(B, C_OUT, H, WP)[:, :, :, :W_]
    return np.ascontiguousarray(o).astype(np.float32)


def kernel(x, W, b):
    x = np.asarray(x, dtype=np.float32)
    W = np.asarray(W, dtype=np.float32)
    b = np.asarray(b, dtype=np.float32)
    nc = _get_program()
    in_maps = _prep_inputs(x, W, b)
    res = run_bass_kernel_spmd(nc, in_maps, list(range(N_CORES)), trace=False)
    full = np.concatenate([res.results[i]["out"] for i in range(N_CORES)], axis=0)
    return _postprocess(full)
